# revision 24
# baseline (speedup 1.0000x reference)
"""Trainium2 Bass kernel for nn_CilLayer: [128,65536,3] f32 -> [128,65536,2] f32.

out0 = -90*(clip(x,-1,1)+1)
out1 = (180/pi)*atan2(z,y) = -(180/pi)*(atan(y/z) - (pi/2)*sign(z))

Final design (tolerance is rel 2e-2 on scale 180 => 3.6 deg absolute;
the bf16 data paths measure ~0.89 deg max, rel ~4.9e-3):
- Host pre-pass per core: planar [3, NPT] bf16 input (x/y/z each
  unit-stride, half the HBM read bytes => 6.29MB/core) and planar
  [2, NPT] bf16 output (host casts/interleaves; all math on device).
  Total device DMA 10.5MB/core => ~24us fabric floor at ~430 GB/s.
- Fused custom-DVE op RECIP_MUL_APPROX_ANT computes y * approx(1/z) in
  one 1x DVE pass (bitwise-NOT seed + one Newton step, ~0.4% worst rel
  err -> ~0.1 deg after atan). Registered via the documented dve_ops
  extension point.
- The existing LN_BWD_DX_ANT custom op computes the whole out1 tail
  (atan - sign*(pi/2)) * -FACTOR in one DVE instruction.
- ACT engine runs only Arctan + Sign (single resident table set, no
  per-chunk table switches).
- Software-pipelined emission with a 2-iteration skew so the in-order
  engines never stall on same-iteration cross-engine producers.
- DMA: all input triggers on the sync-engine HWDGE queue (~350 GB/s
  alone, above the ~240 GB/s compute drain; ACT stays trigger-free);
  outputs on the SWDGE pool queue, last chunks on sync (input done).
- Steady state is compute-bound: DVE ~27us busy, ACT ~24us; plus a
  fixed ~8.6us NRT semaphore-reset postamble after the last byte.

Sharding: batch dim split across 8 NeuronCores (16 batches/core),
purely elementwise, no communication.
"""
import sys
import math

if '/opt/trn_rl_repo' not in sys.path:
    sys.path.insert(0, '/opt/trn_rl_repo')

import numpy as np
import ml_dtypes

B, L = 128, 65536
NCORES = 8
BPC = B // NCORES            # batches per core
NPT = BPC * L                # points per core = 1,048,576
P = 128                      # SBUF partitions
FACTOR = 180.0 / math.pi
BF16 = ml_dtypes.bfloat16

_CACHE = {}


def _get_recip_mul_op():
    """Register (once) a fused y*approx(1/z) custom DVE op.

    body: y0 = bitcast(~z)*c0; y1 = y0*(c1 - z*y0); out = y1 * y
    Seed + one Newton step: ~0.4% worst-case relative error, far inside
    this problem's tolerance. Uses the documented extension point
    (dve_ops.OPS registry); sha pins are filled from the compiler's own
    lowering since this op is new.
    """
    if 'recip_mul' in _CACHE:
        return _CACHE['recip_mul']
    from concourse import dve_ops
    from concourse.dve_spec import AluOp, Bin, C0, C1, Spec, Src0, Src1, lower
    from concourse.dve_uop import DveOpSpec

    name = "RECIP_MUL_APPROX_ANT"
    c0, c1 = dve_ops.RECIP_APPROX_FAST_CONSTS["s0"], \
        dve_ops.RECIP_APPROX_FAST_CONSTS["s1"]

    def _ref(in0, in1, s0, s1, imm2):
        z = np.asarray(in0, dtype=np.float32)
        not_z = (~z.view(np.int32)).view(np.float32)
        y0 = not_z * s0
        y1 = y0 * (s1 - z * y0)
        return (y1 * np.asarray(in1, dtype=np.float32)).astype(np.float32)

    _not_z = Bin(AluOp.BITWISE_NOT, Src0, Src0)
    _y0 = _not_z * C0
    _y1 = _y0 * (C1 - Src0 * _y0)
    op = dve_ops.DveOp(
        name, Spec(body=_y1 * Src1, reference=_ref),
        subdim=False, uops_sha={},
    )
    # register in the module-level tables the compiler reads
    dve_ops.OPS.append(op)
    dve_ops.CUSTOM_DVE_SPECS[name] = op.spec
    dve_ops._SUB_OPCODE_FOR_NAME[name] = (
        dve_ops._CUSTOM_DVE_ROW_BASE + len(dve_ops.OPS) - 1)
    # fill the sha pins from the actual lowering
    for ver in ("v3", "v4"):
        spec = DveOpSpec(
            name=name,
            opcode=dve_ops.get_dve_sub_opcode(name),
            uops=lower(op.spec, ver=ver),
            rd1_en=True,
        )
        op.uops_sha[ver] = spec.sha(ver)
    _CACHE['recip_mul'] = op
    return op


def _build():
    from concourse import mybir, tile, bacc
    from concourse.dve_ops import LN_BWD_DX_ANT
    f32 = mybir.dt.float32
    bf16 = mybir.dt.bfloat16
    AFT = mybir.ActivationFunctionType
    ALU = mybir.AluOpType
    recip_mul = _get_recip_mul_op()

    nc = bacc.Bacc("TRN2", debug=False)
    x = nc.dram_tensor("x", [3, NPT], bf16, kind="ExternalInput").ap()
    o = nc.dram_tensor("o", [2, NPT], bf16, kind="ExternalOutput").ap()

    # ramp length tuned so input delivery (~2.9us/MB early) stays ahead
    # of DVE consumption (~4.2us/MB): enough small/mid chunks of
    # pre-work before the lone 2048 tile is needed
    chunks = [128, 128, 256, 512, 512, 1024, 1024, 1024, 2048, 512,
              512, 256, 128, 128]
    n = len(chunks)
    assert sum(chunks) == NPT // P
    offs = [sum(chunks[:i]) * P for i in range(n)]

    # all input triggers on sync. A/B-tested alternatives that LOSE:
    # splitting early inputs onto the scalar queue (3 active queues
    # starve the pool output stream -> outpool backpressure, +8us) and
    # moving output affines to gpsimd (pool ts is ~5x slower than DVE
    # packed mode and contends with DVE's 2-port SBUF access).
    def in_eng(nc, ci):
        return nc.sync

    # outputs: pool queue early, sync queue tail (input stream done)
    def out_eng(nc, ci):
        return nc.sync if ci >= 9 else nc.gpsimd

    # Software-pipelined emission with a 2-iteration skew: engines run
    # in order, so every emitted instruction must depend only on work
    # from >=1 iteration earlier, or same-iteration same-engine output.
    st = {}
    with tile.TileContext(nc) as tc:
        with tc.tile_pool(name="inp", bufs=5) as inpool, \
             tc.tile_pool(name="outp", bufs=6) as outpool, \
             tc.tile_pool(name="tmp", bufs=4) as tp:
            for it in range(n + 2):
                # ---- drain stage (chunk it-2): o1 fold, o0 affine, store
                if it >= 2:
                    ci = it - 2
                    fd = chunks[ci]
                    s = st.pop(ci)
                    tout = outpool.tile([P, 2 * fd], bf16, tag="out")
                    o0 = tout[:, 0:fd]
                    o1 = tout[:, fd:2 * fd]
                    # o1 = (ta - tsg*(pi/2) - 0) * -FACTOR in one DVE op
                    nc.vector._custom_dve(
                        LN_BWD_DX_ANT, out=o1, in0=s['ta'][:],
                        in1=s['tsg'][:], s0=math.pi / 2.0, s1=0.0,
                        imm2=-FACTOR)
                    # o0 = -90*clip - 90 (bf16 4x-mode DVE)
                    nc.vector.tensor_scalar(
                        o0, s['tclip'][:], -90.0, -90.0, ALU.mult, ALU.add)
                    dst = o[:, offs[ci]:offs[ci] + P * fd].rearrange(
                        "c (p f) -> p c f", p=P)
                    out_eng(nc, ci).dma_start(
                        dst, tout[:].rearrange("p (c f) -> p c f", c=2))

                # ---- mid stage (chunk it-1): arctan
                if 1 <= it <= n:
                    ci = it - 1
                    s = st[ci]
                    ta = tp.tile([P, chunks[ci]], bf16, tag="ta")
                    nc.scalar.activation(ta[:], s['tm'][:], AFT.Arctan)
                    s['ta'] = ta

                # ---- load stage (chunk it): input DMA + first-level ops
                if it < n:
                    ci, fd = it, chunks[it]
                    src = x[:, offs[ci]:offs[ci] + P * fd].rearrange(
                        "c (p f) -> p c f", p=P)
                    tin = inpool.tile([P, 3 * fd], bf16, tag="in")
                    in_eng(nc, ci).dma_start(
                        tin[:].rearrange("p (c f) -> p c f", c=3), src)
                    xv = tin[:, 0:fd]
                    yv = tin[:, fd:2 * fd]
                    zv = tin[:, 2 * fd:3 * fd]
                    tm = tp.tile([P, fd], bf16, tag="tm")
                    nc.vector._custom_dve(
                        recip_mul, out=tm[:], in0=zv, in1=yv,
                        s0=-0.23549792, s1=2.0017324)
                    tclip = tp.tile([P, fd], bf16, tag="tclip")
                    nc.vector.tensor_scalar(
                        tclip[:], xv, 1.0, -1.0, ALU.min, ALU.max)
                    tsg = tp.tile([P, fd], bf16, tag="tsg")
                    nc.scalar.activation(tsg[:], zv, AFT.Sign)
                    st[ci] = {'tm': tm, 'tclip': tclip, 'tsg': tsg}
    nc.compile()
    return nc


def _get_nc():
    if 'nc' not in _CACHE:
        _CACHE['nc'] = _build()
    return _CACHE['nc']


def _in_maps(inputs):
    inputs = np.ascontiguousarray(inputs, dtype=np.float32)
    maps = []
    for c in range(NCORES):
        shard = inputs[c * BPC:(c + 1) * BPC].reshape(NPT, 3)
        planar = shard.T.astype(BF16)  # [3, NPT] C-contiguous bf16
        # z == 0 would NaN the reciprocal seed; +eps reproduces the
        # reference's z -> 0+ limit (psi = 0 for y>0, pi for y<0)
        zrow = planar[2]
        zrow[zrow == 0] = BF16(1e-30)
        maps.append({"x": planar})
    return maps


def kernel(inputs):
    from concourse import bass_utils
    inputs = np.ascontiguousarray(inputs, dtype=np.float32)
    assert inputs.shape == (B, L, 3), inputs.shape
    nc = _get_nc()
    in_maps = _in_maps(inputs)
    res = bass_utils.run_bass_kernel_spmd(nc, in_maps, list(range(NCORES)))
    parts = []
    for c in range(NCORES):
        arr = np.asarray(res.results[c]["o"]).astype(np.float32).reshape(2, NPT)
        parts.append(arr.T.reshape(BPC, L, 2))
    return np.concatenate(parts, axis=0)


# revision 25
# speedup vs baseline: 1.0374x; 1.0374x over previous
"""Trainium2 Bass kernel for nn_CilLayer: [128,65536,3] f32 -> [128,65536,2] f32.

out0 = -90*(clip(x,-1,1)+1)
out1 = (180/pi)*atan2(z,y) = -(180/pi)*(atan(y/z) - (pi/2)*sign(z))

Final design (tolerance is rel 2e-2 on scale 180 => 3.6 deg absolute;
the bf16 data paths measure ~0.89 deg max, rel ~4.9e-3):
- Host pre-pass per core: planar [3, NPT] bf16 input (x/y/z each
  unit-stride, half the HBM read bytes => 6.29MB/core) and planar
  [2, NPT] bf16 output (host casts/interleaves; all math on device).
  Total device DMA 10.5MB/core => ~24us fabric floor at ~430 GB/s.
- Fused custom-DVE op RECIP_MUL_APPROX_ANT computes y * approx(1/z) in
  one 1x DVE pass (bitwise-NOT seed + one Newton step, ~0.4% worst rel
  err -> ~0.1 deg after atan). Registered via the documented dve_ops
  extension point.
- The existing LN_BWD_DX_ANT custom op computes the whole out1 tail
  (atan - sign*(pi/2)) * -FACTOR in one DVE instruction.
- ACT engine runs only Arctan + Sign (single resident table set, no
  per-chunk table switches).
- Software-pipelined emission with a 2-iteration skew so the in-order
  engines never stall on same-iteration cross-engine producers.
- DMA: all input triggers on the sync-engine HWDGE queue (~350 GB/s
  alone, above the ~240 GB/s compute drain; ACT stays trigger-free);
  outputs on the SWDGE pool queue, last chunks on sync (input done).
- Steady state is compute-bound: DVE ~27us busy, ACT ~24us; plus a
  fixed ~8.6us NRT semaphore-reset postamble after the last byte.

Sharding: batch dim split across 8 NeuronCores (16 batches/core),
purely elementwise, no communication.
"""
import sys
import math

if '/opt/trn_rl_repo' not in sys.path:
    sys.path.insert(0, '/opt/trn_rl_repo')

import numpy as np
import ml_dtypes

B, L = 128, 65536
NCORES = 8
BPC = B // NCORES            # batches per core
NPT = BPC * L                # points per core = 1,048,576
P = 128                      # SBUF partitions
FACTOR = 180.0 / math.pi
BF16 = ml_dtypes.bfloat16

_CACHE = {}


def _get_recip_mul_op():
    """Register (once) a fused y*approx(1/z) custom DVE op.

    body: y0 = bitcast(~z)*c0; y1 = y0*(c1 - z*y0); out = y1 * y
    Seed + one Newton step: ~0.4% worst-case relative error, far inside
    this problem's tolerance. Uses the documented extension point
    (dve_ops.OPS registry); sha pins are filled from the compiler's own
    lowering since this op is new.
    """
    if 'recip_mul' in _CACHE:
        return _CACHE['recip_mul']
    from concourse import dve_ops
    from concourse.dve_spec import AluOp, Bin, C0, C1, Spec, Src0, Src1, lower
    from concourse.dve_uop import DveOpSpec

    name = "RECIP_MUL_APPROX_ANT"
    c0, c1 = dve_ops.RECIP_APPROX_FAST_CONSTS["s0"], \
        dve_ops.RECIP_APPROX_FAST_CONSTS["s1"]

    def _ref(in0, in1, s0, s1, imm2):
        z = np.asarray(in0, dtype=np.float32)
        not_z = (~z.view(np.int32)).view(np.float32)
        y0 = not_z * s0
        y1 = y0 * (s1 - z * y0)
        return (y1 * np.asarray(in1, dtype=np.float32)).astype(np.float32)

    _not_z = Bin(AluOp.BITWISE_NOT, Src0, Src0)
    _y0 = _not_z * C0
    _y1 = _y0 * (C1 - Src0 * _y0)
    op = dve_ops.DveOp(
        name, Spec(body=_y1 * Src1, reference=_ref),
        subdim=False, uops_sha={},
    )
    # register in the module-level tables the compiler reads
    dve_ops.OPS.append(op)
    dve_ops.CUSTOM_DVE_SPECS[name] = op.spec
    dve_ops._SUB_OPCODE_FOR_NAME[name] = (
        dve_ops._CUSTOM_DVE_ROW_BASE + len(dve_ops.OPS) - 1)
    # fill the sha pins from the actual lowering
    for ver in ("v3", "v4"):
        spec = DveOpSpec(
            name=name,
            opcode=dve_ops.get_dve_sub_opcode(name),
            uops=lower(op.spec, ver=ver),
            rd1_en=True,
        )
        op.uops_sha[ver] = spec.sha(ver)
    _CACHE['recip_mul'] = op
    return op


def _build():
    from concourse import mybir, tile, bacc
    from concourse.dve_ops import LN_BWD_DX_ANT
    f32 = mybir.dt.float32
    bf16 = mybir.dt.bfloat16
    AFT = mybir.ActivationFunctionType
    ALU = mybir.AluOpType
    recip_mul = _get_recip_mul_op()

    nc = bacc.Bacc("TRN2", debug=False)
    x = nc.dram_tensor("x", [3, NPT], bf16, kind="ExternalInput").ap()
    o = nc.dram_tensor("o", [2, NPT], bf16, kind="ExternalOutput").ap()

    # ramp length tuned so input delivery (~2.9us/MB early) stays ahead
    # of DVE consumption (~4.2us/MB): enough small/mid chunks of
    # pre-work before the lone 2048 tile is needed
    chunks = [128, 128, 256, 512, 1024, 1024, 1024, 2048, 1024, 512,
              256, 128, 128]
    n = len(chunks)
    assert sum(chunks) == NPT // P
    offs = [sum(chunks[:i]) * P for i in range(n)]

    # all input triggers on sync. A/B-tested alternatives that LOSE:
    # splitting early inputs onto the scalar queue (3 active queues
    # starve the pool output stream -> outpool backpressure, +8us) and
    # moving output affines to gpsimd (pool ts is ~5x slower than DVE
    # packed mode and contends with DVE's 2-port SBUF access).
    def in_eng(nc, ci):
        return nc.sync

    # outputs: pool queue early, sync queue tail (input stream done)
    def out_eng(nc, ci):
        return nc.sync if ci >= 9 else nc.gpsimd

    # Software-pipelined emission with a 2-iteration skew: engines run
    # in order, so every emitted instruction must depend only on work
    # from >=1 iteration earlier, or same-iteration same-engine output.
    st = {}
    with tile.TileContext(nc) as tc:
        with tc.tile_pool(name="inp", bufs=5) as inpool, \
             tc.tile_pool(name="outp", bufs=6) as outpool, \
             tc.tile_pool(name="tmp", bufs=4) as tp:
            for it in range(n + 2):
                # ---- drain stage (chunk it-2): o1 fold, o0 affine, store
                if it >= 2:
                    ci = it - 2
                    fd = chunks[ci]
                    s = st.pop(ci)
                    tout = outpool.tile([P, 2 * fd], bf16, tag="out")
                    o0 = tout[:, 0:fd]
                    o1 = tout[:, fd:2 * fd]
                    # o1 = (ta - tsg*(pi/2) - 0) * -FACTOR in one DVE op
                    nc.vector._custom_dve(
                        LN_BWD_DX_ANT, out=o1, in0=s['ta'][:],
                        in1=s['tsg'][:], s0=math.pi / 2.0, s1=0.0,
                        imm2=-FACTOR)
                    # o0 = -90*clip - 90 (bf16 4x-mode DVE)
                    nc.vector.tensor_scalar(
                        o0, s['tclip'][:], -90.0, -90.0, ALU.mult, ALU.add)
                    dst = o[:, offs[ci]:offs[ci] + P * fd].rearrange(
                        "c (p f) -> p c f", p=P)
                    out_eng(nc, ci).dma_start(
                        dst, tout[:].rearrange("p (c f) -> p c f", c=2))

                # ---- mid stage (chunk it-1): arctan
                if 1 <= it <= n:
                    ci = it - 1
                    s = st[ci]
                    ta = tp.tile([P, chunks[ci]], bf16, tag="ta")
                    nc.scalar.activation(ta[:], s['tm'][:], AFT.Arctan)
                    s['ta'] = ta

                # ---- load stage (chunk it): input DMA + first-level ops
                if it < n:
                    ci, fd = it, chunks[it]
                    src = x[:, offs[ci]:offs[ci] + P * fd].rearrange(
                        "c (p f) -> p c f", p=P)
                    tin = inpool.tile([P, 3 * fd], bf16, tag="in")
                    in_eng(nc, ci).dma_start(
                        tin[:].rearrange("p (c f) -> p c f", c=3), src)
                    xv = tin[:, 0:fd]
                    yv = tin[:, fd:2 * fd]
                    zv = tin[:, 2 * fd:3 * fd]
                    tm = tp.tile([P, fd], bf16, tag="tm")
                    nc.vector._custom_dve(
                        recip_mul, out=tm[:], in0=zv, in1=yv,
                        s0=-0.23549792, s1=2.0017324)
                    tclip = tp.tile([P, fd], bf16, tag="tclip")
                    nc.vector.tensor_scalar(
                        tclip[:], xv, 1.0, -1.0, ALU.min, ALU.max)
                    tsg = tp.tile([P, fd], bf16, tag="tsg")
                    nc.scalar.activation(tsg[:], zv, AFT.Sign)
                    st[ci] = {'tm': tm, 'tclip': tclip, 'tsg': tsg}
    nc.compile()
    return nc


def _get_nc():
    if 'nc' not in _CACHE:
        _CACHE['nc'] = _build()
    return _CACHE['nc']


def _in_maps(inputs):
    inputs = np.ascontiguousarray(inputs, dtype=np.float32)
    maps = []
    for c in range(NCORES):
        shard = inputs[c * BPC:(c + 1) * BPC].reshape(NPT, 3)
        planar = shard.T.astype(BF16)  # [3, NPT] C-contiguous bf16
        # z == 0 would NaN the reciprocal seed; +eps reproduces the
        # reference's z -> 0+ limit (psi = 0 for y>0, pi for y<0)
        zrow = planar[2]
        zrow[zrow == 0] = BF16(1e-30)
        maps.append({"x": planar})
    return maps


def kernel(inputs):
    from concourse import bass_utils
    inputs = np.ascontiguousarray(inputs, dtype=np.float32)
    assert inputs.shape == (B, L, 3), inputs.shape
    nc = _get_nc()
    in_maps = _in_maps(inputs)
    res = bass_utils.run_bass_kernel_spmd(nc, in_maps, list(range(NCORES)))
    parts = []
    for c in range(NCORES):
        arr = np.asarray(res.results[c]["o"]).astype(np.float32).reshape(2, NPT)
        parts.append(arr.T.reshape(BPC, L, 2))
    return np.concatenate(parts, axis=0)
